# revision 31
# baseline (speedup 1.0000x reference)
"""GCN (2-layer, sigmoid-gated) on 8 trn2 NeuronCores.

Strategy: node-sharded (graph parallel). Host assigns nodes to 8 cores x
NB blocks of 128 (balanced by indegree), then rebalances source-row
parity (`_balance_parity`) so every (block, src-parity) edge count is
<= 1024, which pins the per-parity tile count at T=8 (minimal gather
rows).  dinv = 1/sqrt(indeg+1) is the symmetric-norm scale.
Device per layer: local GEMM -> g = dinv*u (ACT, per-partition scale) ->
AllGather(g) -> per-block scatter-aggregate: S_b = I^T @ g_b (self loop)
+ sum over edge tiles of M_tile^T @ G_tile accumulated in PSUM, where
G_tile is a 128-row dma_gather from the replicated table and M_tile is a
one-hot(dst) mask built on-device with is_equal against an iota.
Relu/dinv/sigmoid-gates run on the ACT engine per block.  Layer 2 keeps
the table at width OUT=64 and gathers 256B pairs (pair_select), halving
AllGather-2.  Gathers use int16 pair-slot indices (row//2) with the
table viewed as [rows/2, 2*D] and elem_step=2*D; parity picks the base
column.  A tiny AllGather issued before the input loads absorbs the
first-collective rank-sync barrier under the GEMM.
"""
import contextlib
import ctypes
import os
import sys
import types

sys.path.insert(0, "/opt/trn_rl_repo")

import numpy as np
import ml_dtypes

_SO = "/opt/axon/libaxon_pjrt.so"


def _install_shims():
    if "antenv.axon_hooks" not in sys.modules:
        m = types.ModuleType("antenv.axon_hooks")

        def _mk_hook(so_path):
            try:
                lib = ctypes.CDLL(so_path)
            except OSError:
                return None
            if not hasattr(lib, "axon_start_nrt_profile"):
                return None
            lib.axon_start_nrt_profile.argtypes = [
                ctypes.POINTER(ctypes.c_int64), ctypes.c_size_t]
            lib.axon_start_nrt_profile.restype = ctypes.c_int64
            lib.axon_stop_nrt_profile.argtypes = [ctypes.c_char_p]
            lib.axon_stop_nrt_profile.restype = ctypes.c_int64

            @contextlib.contextmanager
            def _hook(output_dir, device_ids):
                import jax
                jax.devices()
                if device_ids:
                    ids = (ctypes.c_int64 * len(device_ids))(*device_ids)
                    rc = lib.axon_start_nrt_profile(ids, len(device_ids))
                else:
                    rc = lib.axon_start_nrt_profile(None, 0)
                if rc != 0:
                    raise RuntimeError(f"axon_start_nrt_profile rc={rc}")
                try:
                    yield
                finally:
                    n = lib.axon_stop_nrt_profile(str(output_dir).encode())
                    if n < 0:
                        raise RuntimeError(f"axon_stop_nrt_profile rc={n}")

            return _hook

        m._hook = _mk_hook(_SO)
        m.set_axon_ntff_profile_hook = lambda h: setattr(m, "_hook", h)
        m.get_axon_ntff_profile_hook = lambda: m._hook
        sys.modules["antenv.axon_hooks"] = m
        try:
            import antenv
            antenv.axon_hooks = m
        except ImportError:
            pass
    import concourse.bass_utils as bu
    bu.upload_artifacts = lambda tmpdir: str(tmpdir)


_install_shims()

import concourse.bacc as bacc
import concourse.tile as tile
import concourse.bass as bass
from concourse import mybir
from concourse.bass_utils import run_bass_kernel_spmd
from concourse.masks import make_identity

P = 128
NCORES = 8
CALL = int(os.environ.get("GAT_CALL", "1024"))  # rows per dma_gather call
SPKT = os.environ.get("GAT_SP", "1") == "1"
GB = int(os.environ.get("GAT_GB", "8"))         # gather pool bufs
MB = int(os.environ.get("GAT_MB", "8"))         # mask pool bufs
NPREP = int(os.environ.get("GAT_PREP", "0"))    # calls pre-emitted per layer
CT = CALL // P              # tiles per call
bf16 = mybir.dt.bfloat16
f32 = mybir.dt.float32
i16 = mybir.dt.int16
i32 = mybir.dt.int32

LAST_EXEC_NS = None         # set after each run when GAT_TRACE=1
_KERNEL_CACHE = {}


# ----------------------------------------------------------------- host side

def _assign_nodes(deg, n, nblocks):
    """Balanced assignment of nodes to nblocks blocks of P slots.
    Returns globalrow[n] (position in the padded 0..nblocks*P space)."""
    order = np.argsort(-deg, kind="stable")
    load = np.zeros(nblocks, dtype=np.int64)
    cnt = np.zeros(nblocks, dtype=np.int64)
    import heapq
    heap = [(0, b) for b in range(nblocks)]
    heapq.heapify(heap)
    pos = np.empty(n, dtype=np.int64)
    for v in order:
        while True:
            l, b = heapq.heappop(heap)
            if cnt[b] < P:
                break
        pos[v] = b * P + cnt[b]
        cnt[b] += 1
        load[b] = l + deg[v]
        if cnt[b] < P:
            heapq.heappush(heap, (load[b], b))
    return pos


def _balance_parity(grow, src, dst, cap=1024, max_iters=1500, seed=1):
    """Swap node pairs within their block so every (block, src-parity) edge
    count is <= cap, letting the per-parity tile count T drop to cap/P."""
    rng = np.random.default_rng(seed)
    n = grow.shape[0]
    eblk = grow[dst] // P
    home = grow // P
    par = grow % 2
    nblocks = int(home.max()) + 1
    outdeg = np.bincount(src, minlength=n).astype(np.float64)
    epar = par[src]
    cnt_e = np.bincount(eblk, weights=(epar == 0).astype(np.float64),
                        minlength=nblocks)
    cnt_o = np.bincount(eblk, weights=(epar == 1).astype(np.float64),
                        minlength=nblocks)
    for _ in range(max_iters):
        viol = (np.maximum(cnt_e - cap, 0) + np.maximum(cnt_o - cap, 0)).sum()
        if viol == 0:
            break
        k_top = 48 if viol > 1000 else (8 if viol > 40 else 3)
        over_e = cnt_e > cap
        at_o = cnt_o >= cap
        over_o = cnt_o > cap
        at_e = cnt_e >= cap
        w_e2o = over_e[eblk].astype(np.float64) - at_o[eblk]
        w_o2e = over_o[eblk].astype(np.float64) - at_e[eblk]
        epar = par[src]
        g_e2o = np.bincount(src, weights=np.where(epar == 0, w_e2o, 0),
                            minlength=n)
        g_o2e = np.bincount(src, weights=np.where(epar == 1, w_o2e, 0),
                            minlength=n)
        gain = (np.where(par == 0, g_e2o, g_o2e) - 0.002 * outdeg
                + rng.uniform(0, 0.001, n))
        key2 = home * 2 + par
        order = np.lexsort((-gain, key2))
        kk, kfirst = np.unique(key2[order], return_index=True)
        best = np.full(nblocks * 2, -1, dtype=np.int64)
        best[kk] = order[kfirst]
        bu = best[0::2]
        bv = best[1::2]
        okm = (bu >= 0) & (bv >= 0)
        pg = np.where(okm, gain[np.clip(bu, 0, n - 1)]
                      + gain[np.clip(bv, 0, n - 1)], -99)
        top = np.argsort(-pg)[:k_top]
        top = top[pg[top] > 0.0]
        if top.size == 0:
            top = np.argsort(-pg)[:1]
            if pg[top[0]] < -0.5:
                break
        u = bu[top]
        v = bv[top]
        gu = grow[u].copy()
        grow[u] = grow[v]
        grow[v] = gu
        par[u] ^= 1
        par[v] ^= 1
        flip = np.concatenate([u, v])
        sel = np.isin(src, flip)
        es = src[sel]
        eb = eblk[sel]
        old_par = 1 - par[es]
        d_e = np.bincount(eb, weights=np.where(old_par == 0, -1.0, 1.0),
                          minlength=nblocks)
        cnt_e += d_e
        cnt_o -= d_e
    return grow


def _wrap_idx16(flat):
    """dma_gather index layout: flat[i] lives at [i % 16 (replicated x8), i // 16]."""
    n = flat.shape[0]
    assert n % 16 == 0
    cols = n // 16
    out = np.zeros((P, cols), dtype=np.uint16)
    out[:16, :] = flat.astype(np.uint16).reshape(cols, 16).T
    out[16:, :] = np.tile(out[:16, :], (7, 1))
    return out.view(np.int16)


def _preprocess(x, edge_index, params):
    N, IN = x.shape
    E = edge_index.shape[1]
    HID = params["W1"].shape[1]
    OUT = params["W2"].shape[1]
    shard = -(-N // (NCORES * P)) * P         # rows per core, multiple of P
    NB = shard // P
    nblocks = NCORES * NB
    NR = nblocks * P                          # padded global rows

    src = np.asarray(edge_index[0], dtype=np.int64)
    dst = np.asarray(edge_index[1], dtype=np.int64)
    deg = np.bincount(dst, minlength=N) + 1.0
    dinv = deg ** -0.5

    grow = _assign_nodes(np.bincount(dst, minlength=N), N, nblocks)
    grow = _balance_parity(grow, src, dst, cap=8 * P)

    gsrc = grow[src]
    gdst = grow[dst]
    core_of = gdst // shard
    block_of = (gdst % shard) // P
    dloc_of = gdst % P
    par = gsrc % 2
    slot = gsrc // 2

    # group edges by (core, block, parity); sort by source address within a
    # group so each gather call's descriptors hit ascending HBM addresses
    key = (core_of * NB + block_of) * 2 + par
    order = np.lexsort((slot, key))
    key_s = key[order]
    slot_s = slot[order]
    dloc_s = dloc_of[order]
    counts = np.bincount(key_s, minlength=nblocks * 2)
    cnt_e = counts[0::2].reshape(NCORES, NB)
    cnt_o = counts[1::2].reshape(NCORES, NB)
    T_e = max(1, int(-(-cnt_e.max() // P)))
    T_o = max(1, int(-(-cnt_o.max() // P)))

    ntile_e = NB * T_e
    ntile_o = NB * T_o
    ntile_e_pad = -(-ntile_e // CT) * CT
    ntile_o_pad = -(-ntile_o // CT) * CT

    starts = np.zeros(nblocks * 2 + 1, dtype=np.int64)
    np.cumsum(counts, out=starts[1:])

    idx_e = np.zeros((NCORES, ntile_e_pad * P), dtype=np.int64)
    idx_o = np.zeros((NCORES, ntile_o_pad * P), dtype=np.int64)
    dl_e = np.full((NCORES, ntile_e_pad * P), 200.0, dtype=np.float32)
    dl_o = np.full((NCORES, ntile_o_pad * P), 200.0, dtype=np.float32)
    for c in range(NCORES):
        for b in range(NB):
            k2 = (c * NB + b) * 2
            s0, s1 = starts[k2], starts[k2 + 1]
            o0 = (b * T_e) * P
            idx_e[c, o0:o0 + (s1 - s0)] = slot_s[s0:s1]
            dl_e[c, o0:o0 + (s1 - s0)] = dloc_s[s0:s1]
            s0, s1 = starts[k2 + 1], starts[k2 + 2]
            o0 = (b * T_o) * P
            idx_o[c, o0:o0 + (s1 - s0)] = slot_s[s0:s1]
            dl_o[c, o0:o0 + (s1 - s0)] = dloc_s[s0:s1]

    # Engine-aware interleave: descs map to SDMA engine slot%16, so give
    # engine e the sorted sub-range [e*CALL/16, (e+1)*CALL/16) of each call
    # instead of an every-16th stride — engines then sweep 16 disjoint
    # ascending HBM regions. Only valid when calls align with block groups.
    if os.environ.get("GAT_EINT", "1") == "1" and \
            T_e * P == CALL and T_o * P == CALL:
        perm = ((np.arange(CALL) % 16) * (CALL // 16)
                + np.arange(CALL) // 16)
        for arr in (idx_e, idx_o, dl_e, dl_o):
            v = arr.reshape(NCORES, -1, CALL)
            v[:] = v[:, :, perm]

    # per-core packed inputs
    xT = np.zeros((NCORES, IN, shard), dtype=ml_dtypes.bfloat16)
    dinv_col = np.zeros((NCORES, P, NB), dtype=np.float32)
    inv_perm_rows = np.empty(N, dtype=np.int64)
    xf = np.asarray(x, dtype=np.float32)
    for c in range(NCORES):
        in_core = (grow // shard) == c
        vids = np.nonzero(in_core)[0]
        local = grow[vids] - c * shard
        xT[c][:, local] = xf[vids].T.astype(ml_dtypes.bfloat16)
        dinv_col[c][local % P, local // P] = dinv[vids]
        inv_perm_rows[vids] = grow[vids]

    meta = dict(N=N, E=E, IN=IN, HID=HID, OUT=OUT, shard=shard, NB=NB,
                NR=NR, T_e=T_e, T_o=T_o,
                ntile_e_pad=ntile_e_pad, ntile_o_pad=ntile_o_pad,
                ab1=float(np.asarray(params["ab1"]).reshape(-1)[0]),
                ab2=float(np.asarray(params["ab2"]).reshape(-1)[0]))

    b1 = np.asarray(params["b1"]); b2 = np.asarray(params["b2"])
    assert np.all(b1 == 0) and np.all(b2 == 0), "nonzero conv bias unsupported"

    in_maps = []
    for c in range(NCORES):
        in_maps.append({
            "xT": np.ascontiguousarray(xT[c]),
            "dinv": np.ascontiguousarray(dinv_col[c]),
            "idx_e": _wrap_idx16(idx_e[c]),
            "idx_o": _wrap_idx16(idx_o[c]),
            "dl_e": np.ascontiguousarray(dl_e[c].reshape(ntile_e_pad, P).T.astype(ml_dtypes.bfloat16)),
            "dl_o": np.ascontiguousarray(dl_o[c].reshape(ntile_o_pad, P).T.astype(ml_dtypes.bfloat16)),
            "W1": np.asarray(params["W1"], np.float32).astype(ml_dtypes.bfloat16),
            "W2p": np.pad(np.asarray(params["W2"], np.float32), ((0, 0), (0, 0))
                          ).astype(ml_dtypes.bfloat16),
            "aw1": np.asarray(params["aw1"], np.float32).astype(ml_dtypes.bfloat16),
            "aw2": np.asarray(params["aw2"], np.float32).astype(ml_dtypes.bfloat16),
        })
    return in_maps, meta, inv_perm_rows


# --------------------------------------------------------------- device side

def _build(meta):
    IN, HID, OUT = meta["IN"], meta["HID"], meta["OUT"]
    NB, shard, NR = meta["NB"], meta["shard"], meta["NR"]
    T_e, T_o = meta["T_e"], meta["T_o"]
    nte, nto = meta["ntile_e_pad"], meta["ntile_o_pad"]
    KIN = IN // P
    RG = [list(range(NCORES))]

    nc = bacc.Bacc(num_devices=NCORES, num_swdge_queues=4)
    xT_d = nc.dram_tensor("xT", [IN, shard], bf16, kind="ExternalInput")
    dinv_d = nc.dram_tensor("dinv", [P, NB], f32, kind="ExternalInput")
    idx_e_d = nc.dram_tensor("idx_e", [P, nte * 8], i16, kind="ExternalInput")
    idx_o_d = nc.dram_tensor("idx_o", [P, nto * 8], i16, kind="ExternalInput")
    dl_e_d = nc.dram_tensor("dl_e", [P, nte], bf16, kind="ExternalInput")
    dl_o_d = nc.dram_tensor("dl_o", [P, nto], bf16, kind="ExternalInput")
    W1_d = nc.dram_tensor("W1", [IN, HID], bf16, kind="ExternalInput")
    W2_d = nc.dram_tensor("W2p", [HID, OUT], bf16, kind="ExternalInput")
    aw1_d = nc.dram_tensor("aw1", [HID, 1], bf16, kind="ExternalInput")
    aw2_d = nc.dram_tensor("aw2", [OUT, 1], bf16, kind="ExternalInput")
    out_d = nc.dram_tensor("out", [shard, OUT], f32, kind="ExternalOutput")

    with tile.TileContext(nc) as tc:
        with contextlib.ExitStack() as ctx:
            sp = ctx.enter_context(tc.tile_pool(name="sbuf", bufs=1))
            gp = ctx.enter_context(tc.tile_pool(name="gath", bufs=GB))
            mp = ctx.enter_context(tc.tile_pool(name="mtiles", bufs=MB))
            ep = ctx.enter_context(tc.tile_pool(name="epil", bufs=3))
            pp = ctx.enter_context(tc.tile_pool(name="psum", bufs=2, space="PSUM"))
            dp = ctx.enter_context(tc.tile_pool(name="dram", bufs=1, space="DRAM"))

            # ---- early tiny collective: absorbs the rank-sync barrier while
            # input DMAs and the local GEMM run, so AG1 starts without it.
            # The payload is an uninitialized dram tile (contents irrelevant,
            # output unused) so the trigger has no producer dependency and
            # fires at t~0.
            bar_in = dp.tile([P, 8], bf16, name="bar_in")
            bar_out = dp.tile([P * NCORES, 8], bf16, name="bar_out",
                              addr_space="Shared")
            nc.gpsimd.collective_compute(
                "AllGather", mybir.AluOpType.bypass,
                ins=[bar_in[:]], outs=[bar_out[:]], replica_groups=RG)

            # ---- resident loads
            xT_sb = sp.tile([P, KIN, shard], bf16, name="xT_sb")
            nc.sync.dma_start(out=xT_sb[:], in_=xT_d[:].rearrange(
                "(k p) s -> p k s", p=P))
            dinv_sb = sp.tile([P, NB], f32, name="dinv_sb")
            nc.sync.dma_start(out=dinv_sb[:], in_=dinv_d[:])
            idx_e_sb = sp.tile([P, nte * 8], i16, name="idx_e_sb")
            nc.sync.dma_start(out=idx_e_sb[:], in_=idx_e_d[:])
            idx_o_sb = sp.tile([P, nto * 8], i16, name="idx_o_sb")
            nc.sync.dma_start(out=idx_o_sb[:], in_=idx_o_d[:])
            dl_e_sb = sp.tile([P, nte], bf16, name="dl_e_sb")
            nc.sync.dma_start(out=dl_e_sb[:], in_=dl_e_d[:])
            dl_o_sb = sp.tile([P, nto], bf16, name="dl_o_sb")
            nc.sync.dma_start(out=dl_o_sb[:], in_=dl_o_d[:])
            W1_sb = sp.tile([P, KIN, HID], bf16, name="W1_sb")
            nc.sync.dma_start(out=W1_sb[:], in_=W1_d[:].rearrange(
                "(k p) h -> p k h", p=P))
            W2_sb = sp.tile([P, OUT], bf16, name="W2_sb")
            nc.sync.dma_start(out=W2_sb[:], in_=W2_d[:])
            aw1_sb = sp.tile([P, 1], bf16, name="aw1_sb")
            nc.sync.dma_start(out=aw1_sb[:], in_=aw1_d[:])
            aw2_sb = sp.tile([OUT, 1], bf16, name="aw2_sb")
            nc.sync.dma_start(out=aw2_sb[:], in_=aw2_d[:])

            TMX = max(T_e, T_o)
            iota_i = sp.tile([P, TMX, P], i32, name="iota_i")
            nc.gpsimd.iota(iota_i[:], pattern=[[0, TMX], [1, P]], base=0,
                           channel_multiplier=0)
            iota_bf = sp.tile([P, TMX, P], bf16, name="iota_bf")
            nc.vector.tensor_copy(iota_bf[:], iota_i[:])
            ident = sp.tile([P, P], bf16, name="ident")
            make_identity(nc, ident[:])
            identf = sp.tile([P, P], f32, name="identf")
            make_identity(nc, identf[:])

            g1_sb = sp.tile([P, NB, HID], bf16, name="g1_sb")
            g2_sb = sp.tile([P, NB, OUT], bf16, name="g2_sb")
            out_sb = sp.tile([P, NB, OUT], f32, name="out_sb")

            g1_loc = dp.tile([shard, HID], bf16, name="g1_loc")
            g2_loc = dp.tile([shard, OUT], bf16, name="g2_loc")
            g1_full = dp.tile([NR, HID], bf16, name="g1_full",
                              addr_space="Shared")
            g2_full = dp.tile([NR, OUT], bf16, name="g2_full",
                              addr_space="Shared")

            # ---- layer-1 GEMM + scale; per-block DMA so AG1 triggers as
            # soon as the last block lands instead of after one big copy.
            g1_loc_v = g1_loc[:].rearrange("(b p) h -> p b h", p=P)
            for b in range(NB):
                u_ps = pp.tile([P, HID], f32, name="u_ps", tag="tr_ps")
                for k in range(KIN):
                    nc.tensor.matmul(
                        out=u_ps[:], lhsT=xT_sb[:, k, b * P:(b + 1) * P],
                        rhs=W1_sb[:, k, :], start=(k == 0), stop=(k == KIN - 1))
                nc.scalar.activation(
                    g1_sb[:, b, :], u_ps[:],
                    mybir.ActivationFunctionType.Copy,
                    scale=dinv_sb[:, b:b + 1])
                nc.sync.dma_start(out=g1_loc_v[:, b, :], in_=g1_sb[:, b, :])

            nc.gpsimd.collective_compute(
                "AllGather", mybir.AluOpType.bypass,
                ins=[g1_loc[:]], outs=[g1_full[:]], replica_groups=RG)

            # ---- aggregation + epilogue helper
            def aggregate(layer, g_full_ap, width, idx_sb, dl_sb, T_e_, T_o_,
                          nte_, nto_, self_sb, pair_select=False):
                """Returns list of per-block S psum tiles (incl. self term)."""
                pairs = g_full_ap.rearrange("(n two) d -> n (two d)", two=2)
                gw = 2 * width if pair_select else width
                calls = {}

                def emit_gather(par, c, prepare_only=False):
                    isb = idx_sb[par]
                    t = gp.tile([P, CT, gw], bf16, name=f"g{layer}c",
                                tag=f"g{layer}c")
                    src = pairs[:, :] if pair_select else \
                        pairs[:, (par * width):(par * width) + width]
                    sem = nc.alloc_semaphore(
                        f"gprep{layer}_{par}_{c}") if prepare_only else None
                    nc.gpsimd.dma_gather(
                        t[:], src,
                        isb[:, c * (CALL // 16):(c + 1) * (CALL // 16)],
                        CALL, CALL, gw, elem_step=2 * width,
                        single_packet=SPKT, queue_num=(c * 2 + par) % 4,
                        prepare_only=prepare_only, sem=sem)
                    calls[(par, c)] = t
                    return t

                # pre-emit descriptors for the first calls so the Q7 engines
                # work during the AllGather; triggers carry the data dep.
                prep_queues = set()
                for k in range(NPREP):
                    c, par = divmod(k, 2)
                    if (par == 0 and c * CT * P < nte_ * P) or \
                       (par == 1 and c * CT * P < nto_ * P):
                        emit_gather(par, c, prepare_only=True)
                        prep_queues.add((c * 2 + par) % 4)
                for q in sorted(prep_queues):
                    nc.gpsimd.trigger_dma(count=None, queue_num=q)

                def get_call(par, c):
                    key = (par, c)
                    if key in calls:
                        return calls[key]
                    emit_gather(par, c)
                    return calls[key]

                s_tiles = []
                for b in range(NB):
                    s_ps = pp.tile([P, width], f32, name=f"s{layer}_ps",
                                   tag="s_ps", bufs=4, padded_shape=[P, P])
                    ms = []
                    for par, T_ in ((0, T_e_), (1, T_o_)):
                        dl = dl_sb[par]
                        m_t = mp.tile([P, T_, P], bf16, name=f"m{layer}_{par}",
                                      tag=f"m{layer}_{par}",
                                      padded_shape=[P, max(T_e_, T_o_), P])
                        nc.vector.tensor_tensor(
                            out=m_t[:], in0=iota_bf[:, :T_, :],
                            in1=dl[:, b * T_:(b + 1) * T_].unsqueeze(2)
                                .to_broadcast([P, T_, P]),
                            op=mybir.AluOpType.is_equal)
                        ms.append(m_t)
                    # self-loop term: S += I^T @ g_local[b]
                    nc.tensor.matmul(
                        out=s_ps[:], lhsT=ident[:],
                        rhs=self_sb[:, b, :width],
                        start=True, stop=False)
                    nslots = T_e_ + T_o_
                    for s in range(nslots):
                        par, sl = (0, s) if s < T_e_ else (1, s - T_e_)
                        k = b * (T_e_ if par == 0 else T_o_) + sl
                        c, j = divmod(k, CT)
                        gt = get_call(par, c)
                        if pair_select:
                            rhs = gt[:, j, par * width:(par + 1) * width]
                        else:
                            rhs = gt[:, j, :width]
                        nc.tensor.matmul(
                            out=s_ps[:], lhsT=ms[par][:, sl, :],
                            rhs=rhs,
                            start=False, stop=(s == nslots - 1))
                    s_tiles.append(s_ps)
                return s_tiles

            # ---- layer-1 aggregation + epilogue + layer-2 GEMM
            s1 = aggregate(1, g1_full[:], HID, (idx_e_sb, idx_o_sb),
                           (dl_e_sb, dl_o_sb), T_e, T_o, nte, nto, g1_sb)
            g2_loc_v = g2_loc[:].rearrange("(b p) h -> p b h", p=P)
            for b in range(NB):
                h1 = ep.tile([P, HID], bf16, name="h1", tag="h1")
                nc.scalar.activation(
                    h1[:], s1[b][:], mybir.ActivationFunctionType.Relu,
                    scale=dinv_sb[:, b:b + 1])
                tr_ps = pp.tile([P, HID], bf16, name="tr_ps", tag="tr_ps")
                nc.tensor.transpose(out=tr_ps[:], in_=h1[:], identity=ident[:])
                h1T = ep.tile([P, HID], bf16, name="h1T", tag="h1T")
                nc.scalar.activation(h1T[:], tr_ps[:],
                                     mybir.ActivationFunctionType.Copy)
                gt_ps = pp.tile([P, 1], f32, name="gt_ps", tag="gt_ps")
                nc.tensor.matmul(out=gt_ps[:], lhsT=h1T[:], rhs=aw1_sb[:],
                                 start=True, stop=True)
                gate = ep.tile([P, 1], f32, name="gate", tag="gate")
                nc.scalar.activation(gate[:], gt_ps[:],
                                     mybir.ActivationFunctionType.Sigmoid,
                                     bias=meta["ab1"])
                gated = ep.tile([P, HID], bf16, name="gated", tag="gated")
                nc.vector.tensor_tensor(
                    out=gated[:], in0=h1[:],
                    in1=gate[:].to_broadcast([P, HID]),
                    op=mybir.AluOpType.mult)
                tr2_ps = pp.tile([P, HID], bf16, name="tr2_ps", tag="tr_ps")
                nc.tensor.transpose(out=tr2_ps[:], in_=gated[:],
                                    identity=ident[:])
                gatedT = ep.tile([P, HID], bf16, name="gatedT", tag="gatedT")
                nc.scalar.activation(gatedT[:], tr2_ps[:],
                                     mybir.ActivationFunctionType.Copy)
                u2_ps = pp.tile([P, OUT], f32, name="u2_ps", tag="gt_ps")
                nc.tensor.matmul(out=u2_ps[:], lhsT=gatedT[:], rhs=W2_sb[:],
                                 start=True, stop=True)
                nc.scalar.activation(
                    g2_sb[:, b, :], u2_ps[:],
                    mybir.ActivationFunctionType.Copy,
                    scale=dinv_sb[:, b:b + 1])
                nc.sync.dma_start(out=g2_loc_v[:, b, :], in_=g2_sb[:, b, :])

            nc.gpsimd.collective_compute(
                "AllGather", mybir.AluOpType.bypass,
                ins=[g2_loc[:]], outs=[g2_full[:]], replica_groups=RG)

            # ---- layer-2 aggregation + epilogue
            s2 = aggregate(2, g2_full[:], OUT, (idx_e_sb, idx_o_sb),
                           (dl_e_sb, dl_o_sb), T_e, T_o, nte, nto, g2_sb,
                           pair_select=True)
            out_d_v = out_d[:].rearrange("(b p) h -> p b h", p=P)
            for b in range(NB):
                o2 = ep.tile([P, OUT], f32, name="o2", tag="h1")
                nc.scalar.activation(
                    o2[:], s2[b][:], mybir.ActivationFunctionType.Copy,
                    scale=dinv_sb[:, b:b + 1])
                tr3_ps = pp.tile([OUT, P], f32, name="tr3_ps", tag="tr_ps")
                nc.tensor.transpose(out=tr3_ps[:], in_=o2[:],
                                    identity=identf[:])
                o2T = ep.tile([OUT, P], bf16, name="o2T", tag="h1T")
                nc.scalar.activation(o2T[:], tr3_ps[:],
                                     mybir.ActivationFunctionType.Copy)
                gt2_ps = pp.tile([P, 1], f32, name="gt2_ps", tag="gt_ps")
                nc.tensor.matmul(out=gt2_ps[:], lhsT=o2T[:], rhs=aw2_sb[:],
                                 start=True, stop=True)
                gate2 = ep.tile([P, 1], f32, name="gate2", tag="gate")
                nc.scalar.activation(gate2[:], gt2_ps[:],
                                     mybir.ActivationFunctionType.Sigmoid,
                                     bias=meta["ab2"])
                nc.vector.tensor_tensor(
                    out=out_sb[:, b, :], in0=o2[:],
                    in1=gate2[:].to_broadcast([P, OUT]),
                    op=mybir.AluOpType.mult)
                nc.sync.dma_start(out=out_d_v[:, b, :], in_=out_sb[:, b, :])
    nc.compile()
    return nc


# ----------------------------------------------------------------- interface

def kernel(x, edge_index, W1, b1, W2, b2, aw1, ab1, aw2, ab2):
    global LAST_EXEC_NS
    params = dict(W1=W1, b1=b1, W2=W2, b2=b2, aw1=aw1, ab1=ab1,
                  aw2=aw2, ab2=ab2)
    in_maps, meta, grow = _preprocess(x, edge_index, params)

    ck = (meta["N"], meta["E"], meta["IN"], meta["HID"], meta["OUT"],
          meta["T_e"], meta["T_o"], meta["ab1"], meta["ab2"],
          CALL, GB, MB, SPKT, NPREP)
    nc = _KERNEL_CACHE.get(ck)
    if nc is None:
        nc = _build(meta)
        _KERNEL_CACHE[ck] = nc

    trace = os.environ.get("GAT_TRACE", "0") == "1"
    res = run_bass_kernel_spmd(nc, in_maps, core_ids=list(range(NCORES)),
                               trace=trace)
    LAST_EXEC_NS = res.exec_time_ns
    full = np.concatenate([res.results[c]["out"] for c in range(NCORES)],
                          axis=0)
    out = full[grow]
    return np.ascontiguousarray(out.astype(np.float32))

